# revision 16
# baseline (speedup 1.0000x reference)
"""ConvGRU Trainium2 kernel.

Full inputs -> 8-core SPMD Bass kernel -> full output.

Sharding: 8 cores = 4 batch elements x 2 H-halves. Each core owns 32 rows
of H and redundantly computes a "growing halo" (rows 32..32+e, e = T-t)
so the T=16 recurrence needs no cross-core communication. Bottom halves
are vertically flipped on the host (x rows and W ky taps) so every core
runs the identical program: owned rows 0..31, halo extending downward,
zero-pad above row 0.

Per step t the fused conv gi+gh accumulates into two PSUM tiles
  p1 = [i_n | r_pre], p2 = [h_n | z_pre]
from two matmul families per tile (6 + 6 = 12 MMs/chunk vs 18 in the
all-bf16 version):
  - x-side: bf16, ky stacked in partitions (x@ky0|x@ky1|x@ky2 = 96
    rows, physically row-shifted copies loaded by DMA), kx via 3 view
    offsets -> 3 MMs of K=96 per tile.
  - h-side: fp8e4m3 DoubleRow, ky stacked AND channel-paired: partition
    32*ky+cp holds planes (ch=2cp, 2cp+1) shifted by ky; one DR MM
    contracts virtual K=192 (2 fp8 weights/cell) -> 3 MMs per tile.
fp8 on the h-side only keeps rel err ~1.2e-2 (x-side fp8 would blow the
2e-2 budget; bf16 x-side contributes ~2.8e-3).

Elementwise GRU math runs on partitions 64..127 (PSUM operands may be
read at a different base partition than SBUF operands; SBUF-SBUF pairs
stay aligned). h state: f32 master for output + fp8 copy DMA'd into the
next step's DR conv buffer (3 ky destinations).
"""

import os
import sys

sys.path.insert(0, "/opt/trn_rl_repo")

import numpy as np

T, CIN, HID, H, W = 16, 32, 64, 64, 64
B = 4
NCORES = 8
OWN = 32           # owned H rows per core
XR = 48            # x slice rows fed to each core (owned + max halo + 1)
PR = 49            # padded rows
PC = 66            # padded cols
CHUNK = 8          # output rows per chunk (8*64 = 512 = one PSUM bank)

_CACHE = {}
KERNEL_STATS = {}


def _n_rows(t):
    # valid h_t rows needed: owned + halo that future steps consume
    return OWN + (T - t)


def _build():
    import concourse.bacc as bacc
    import concourse.mybir as mybir
    from concourse import tile

    dt = mybir.dt
    AF = mybir.ActivationFunctionType
    DR = mybir.MatmulPerfMode.DoubleRow

    nc = bacc.Bacc("TRN2", target_bir_lowering=False, debug=False,
                   num_devices=NCORES)
    # x pre-padded + ky-stacked on host -> whole-tile contiguous DMA
    xin = nc.dram_tensor("xin", [T, 96, PR, PC], dt.bfloat16,
                         kind="ExternalInput")
    # x-side weights: [96, (tile j, kx), 128] bf16
    wx = nc.dram_tensor("wx", [96, 6 * 128], dt.bfloat16,
                        kind="ExternalInput")
    # h-side DR weights: [96, (tile j, kx), 2, 128] fp8
    wh = nc.dram_tensor("wh", [96, 6 * 2 * 128], dt.float8e4,
                        kind="ExternalInput")
    # chunk-major so each store is contiguous per partition
    out = nc.dram_tensor("out", [T, OWN // CHUNK, HID, CHUNK, W], dt.float32,
                         kind="ExternalOutput")

    with tile.TileContext(nc) as tc:
        with tc.tile_pool(name="const", bufs=1) as const, \
             tc.tile_pool(name="state", bufs=1) as state, \
             tc.tile_pool(name="work", bufs=3) as work, \
             tc.tile_pool(name="psum", bufs=4, space="PSUM") as psum:

            wxs = const.tile([96, 6, 128], dt.bfloat16, tag="wxs")
            whs = const.tile([96, 6, 2, 128], dt.float8e4, tag="whs")
            nc.sync.dma_start(wxs[:], wx.rearrange("p (a m) -> p a m", m=128))
            nc.sync.dma_start(whs[:],
                              wh.rearrange("p (a i m) -> p a i m", i=2,
                                           m=128))

            # x conv buffers, ky-stacked, rotated mod 3
            xb = [state.tile([96, PR, PC], dt.bfloat16, tag=f"xb{i}",
                             name=f"xb{i}")
                  for i in range(3)]
            for i in range(3):
                nc.gpsimd.memset(xb[i][:], 0.0)

            # h DR conv buffers: partition 32*ky+cp holds ch (cp, cp+32)
            hB = [state.tile([96, 2, PR, PC], dt.float8e4, tag=f"h8{i}",
                             name=f"h8{i}")
                  for i in range(2)]
            # hB[0] is read as the zero initial state; hB[1] only needs
            # its never-written ky=0 pad row zeroed (cols/pads elsewhere
            # are covered by the full-width h writes)
            nc.gpsimd.memset(hB[0][:], 0.0)
            nc.gpsimd.memset(hB[1][0:32, :, 0:1, :], 0.0)

            # f32 hidden state (upper 64 partitions), ping-pong
            NH = _n_rows(1)  # 47
            hf = [state.tile([128, NH * W], dt.float32, tag=f"hf{i}",
                             name=f"hf{i}")
                  for i in range(2)]
            nc.vector.memset(hf[0][64:128, :], 0.0)

            HW_ = CHUNK * W

            def emit_mms(t, ci):
                nt = _n_rows(t)
                cx = xb[(t - 1) % 3]
                ch = hB[(t - 1) % 2]
                r0 = ci * CHUNK
                nr = min(CHUNK, nt - r0)
                N = nr * W
                # merged psum: cols [0:512) = p1 = [i_n | r],
                #              cols [512:1024) = p2 = [h_n | z]
                pm = psum.tile([128, 2 * HW_], dt.float32, tag="pm")
                for j in (0, 1):
                    o = j * HW_
                    for kx in range(3):
                        nc.tensor.matmul(
                            pm[:, o:o + N], wxs[:, j * 3 + kx, :],
                            cx[0:96, r0:r0 + nr, kx:kx + W],
                            start=(kx == 0), stop=False)
                for j in (0, 1):
                    o = j * HW_
                    for kx in range(3):
                        nc.tensor.matmul(
                            pm[:, o:o + N], whs[:, j * 3 + kx, :, :],
                            ch[0:96, 0:2, r0:r0 + nr, kx:kx + W],
                            start=False, stop=(kx == 2), perf_mode=DR)
                return pm

            def emit_e1(t, ci, pm):
                # A = pm[:,0:512] = [r|z]; B = pm[:,512:1024] = [h_n|i_n]
                nt = _n_rows(t)
                r0 = ci * CHUNK
                nr = min(CHUNK, nt - r0)
                N = nr * W
                rz = work.tile([128, HW_], dt.float32, tag="rz")
                # one 128-partition sigmoid covers r and z
                nc.scalar.activation(rz[:, :N], pm[:, :N], AF.Sigmoid)
                t_rn = work.tile([128, HW_], dt.float32, tag="t_rn")
                # r * h_n  (both at partitions 0:64)
                nc.vector.tensor_mul(t_rn[0:64, :N], rz[0:64, :N],
                                     pm[0:64, HW_:HW_ + N])
                # + i_n (PSUM upper, cross-base read)
                nc.vector.tensor_add(t_rn[0:64, :N], t_rn[0:64, :N],
                                     pm[64:128, HW_:HW_ + N])
                return rz, t_rn

            def emit_e2(t, ci, rz, t_rn):
                nt = _n_rows(t)
                nh = hB[t % 2]
                hprev = hf[(t - 1) % 2]
                hcur = hf[t % 2]
                r0 = ci * CHUNK
                nr = min(CHUNK, nt - r0)
                N = nr * W
                n_t = work.tile([128, HW_], dt.float32, tag="n_t")
                # cross-base dst: n lands on upper partitions where the
                # h-update path (z, hprev, hcur) lives
                nc.scalar.activation(n_t[64:128, :N], t_rn[0:64, :N],
                                     AF.Tanh)
                d_t = work.tile([128, HW_], dt.float32, tag="d_t")
                nc.gpsimd.tensor_sub(d_t[64:128, :N],
                                     hprev[64:128, r0 * W:r0 * W + N],
                                     n_t[64:128, :N])
                nc.vector.tensor_mul(d_t[64:128, :N],
                                     rz[64:128, :N], d_t[64:128, :N])
                # h_new = n + z*(h - n) -> persistent f32 state
                nc.gpsimd.tensor_add(hcur[64:128, r0 * W:r0 * W + N],
                                     n_t[64:128, :N], d_t[64:128, :N])
                # fp8 convert for the next DR conv input; 66-wide staging
                # with zero edge cols -> contiguous nr*66 runs per
                # partition on the conv-buffer writes
                hb = work.tile([128, CHUNK, PC], dt.float8e4, tag="hb")
                nc.vector.memset(hb[64:128, 0:nr, 0:1], 0.0)
                nc.vector.memset(hb[64:128, 0:nr, 65:66], 0.0)
                nc.scalar.copy(
                    hb[64:128, 0:nr, 1:65],
                    hcur[64:128, r0 * W:r0 * W + N].rearrange(
                        "p (r c) -> p r c", c=W))
                # group ky: buf row d holds h row d-1+ky; DR pair
                # (plane i) = ch cp + 32*i -> contiguous src ranges
                for i in range(2):
                    src8 = hb[64 + 32 * i:96 + 32 * i, 0:nr, :]
                    nc.sync.dma_start(
                        nh[0:32, i, 1 + r0:1 + r0 + nr, :], src8)
                    nc.sync.dma_start(
                        nh[32:64, i, r0:r0 + nr, :], src8)
                    if r0 == 0:
                        nc.sync.dma_start(
                            nh[64:96, i, 0:nr - 1, :],
                            hb[64 + 32 * i:96 + 32 * i, 1:nr, :])
                    else:
                        nc.sync.dma_start(
                            nh[64:96, i, r0 - 1:r0 - 1 + nr, :], src8)
                if r0 < OWN:
                    src_f = hcur[64:128, r0 * W:r0 * W + N]
                    nc.sync.dma_start(
                        out[t - 1, ci].rearrange("c r w -> c (r w)"), src_f)

            # software-pipelined emission: MMs(k) | E1(k) | E2(k-1)
            sched = []
            for t in range(1, T + 1):
                for ci in range((_n_rows(t) + CHUNK - 1) // CHUNK):
                    sched.append((t, ci))
            pend = None
            for k, (t, ci) in enumerate(sched):
                if ci == 0:
                    if t == 1:
                        nc.sync.dma_start(xb[0][:], xin[0])
                    if t < T:
                        nc.sync.dma_start(xb[t % 3][:], xin[t])
                pm = emit_mms(t, ci)
                e1 = emit_e1(t, ci, pm)
                if pend is not None:
                    emit_e2(*pend)
                pend = (t, ci) + e1
            emit_e2(*pend)

    nc.compile()
    return nc


def _prep_inputs(x, W_i, W_h):
    import ml_dtypes

    bf16 = ml_dtypes.bfloat16
    fp8 = ml_dtypes.float8_e4m3
    in_maps = []
    for c in range(NCORES):
        b, half = divmod(c, 2)
        xs = x[b]                      # [T, CIN, H, W]
        Wi, Wh = W_i, W_h
        if half == 1:
            xs = xs[:, :, ::-1, :]
            Wi = W_i[:, :, ::-1, :]
            Wh = W_h[:, :, ::-1, :]
        xs = np.asarray(xs[:, :, :XR, :], np.float32)
        # pre-padded + ky-stacked x image: group ky row d = x row d-1+ky
        xp = np.zeros((T, 96, PR, PC), np.float32)
        xp[:, 0:32, 1:49, 1:65] = xs
        xp[:, 32:64, 0:48, 1:65] = xs
        xp[:, 64:96, 0:47, 1:65] = xs[:, :, 1:48, :]
        xp = xp.astype(bf16)

        # x-side: wx[32*ky+c, j, kx, m]
        wx = np.zeros((3, 32, 2, 3, 128), np.float32)
        # h-side DR: wh[32*ky+cp, j, kx, i, m], ch = 2*cp+i
        wh = np.zeros((3, 32, 2, 3, 2, 128), np.float32)
        for ky in range(3):
            for kx in range(3):
                wik = Wi[:, :, ky, kx]       # [192, 32]
                whk = Wh[:, :, ky, kx]       # [192, 64]
                # j=0 -> A = [r | z]
                wx[ky, :, 0, kx, 0:64] = wik[0:64].T
                wx[ky, :, 0, kx, 64:128] = wik[64:128].T
                # j=1 -> B = [h_n | i_n]
                wx[ky, :, 1, kx, 64:128] = wik[128:192].T
                # [cp, i, gate-ch], pair plane i holds ch = cp + 32*i
                whp = whk.T.reshape(2, 32, 192).transpose(1, 0, 2)
                wh[ky, :, 0, kx, :, 0:64] = whp[:, :, 0:64]
                wh[ky, :, 0, kx, :, 64:128] = whp[:, :, 64:128]
                wh[ky, :, 1, kx, :, 0:64] = whp[:, :, 128:192]
        wx = wx.transpose(0, 1, 2, 3, 4).reshape(96, 2, 3, 128)
        wx = np.ascontiguousarray(wx.reshape(96, 6 * 128)).astype(bf16)
        wh = wh.reshape(96, 2, 3, 2, 128)
        wh = np.ascontiguousarray(wh.reshape(96, 6 * 2 * 128)).astype(fp8)
        in_maps.append({"xin": xp, "wx": wx, "wh": wh})
    return in_maps


def kernel(x, W_i, W_h):
    from concourse.bass_utils import run_bass_kernel_spmd

    x = np.asarray(x, dtype=np.float32)
    W_i = np.asarray(W_i, dtype=np.float32)
    W_h = np.asarray(W_h, dtype=np.float32)

    if "nc" not in _CACHE:
        _CACHE["nc"] = _build()
    nc = _CACHE["nc"]

    in_maps = _prep_inputs(x, W_i, W_h)
    trace = bool(os.environ.get("BASS_TRACE"))
    res = run_bass_kernel_spmd(nc, in_maps, list(range(NCORES)), trace=trace)
    KERNEL_STATS["exec_time_ns"] = res.exec_time_ns
    KERNEL_STATS["trace"] = res.instructions_and_trace

    y = np.empty((B, T, HID, H, W), np.float32)
    for c in range(NCORES):
        b, half = divmod(c, 2)
        oc = res.results[c]["out"]     # [T, 4, HID, CHUNK, W]
        oc = np.asarray(oc).transpose(0, 2, 1, 3, 4).reshape(T, HID, OWN, W)
        if half == 0:
            y[b, :, :, 0:OWN, :] = oc
        else:
            y[b, :, :, OWN:H, :] = oc[:, :, ::-1, :]
    return y
